# revision 22
# baseline (speedup 1.0000x reference)
"""Masked cross-attention (B=4, NQ=100, HW=4096, D=1024, H=16) on 8 TRN2 cores.

Sharding: kv rows (keys) are split 8 ways; each core runs LayerNorm + K/V
projection on its 512-key slice per batch, computes unnormalized partial
attention for all (b, h) against its keys, all-reduces the softmax
denominators on device, normalizes, and computes a partial out-projection.
The host sums the 8 partial outputs and adds the folded bias.

Schedule: the kernel is split into per-batch A-blocks (kv LN/transpose,
K-projection, scores, exp*mask, denominators) and B-blocks (V-projection,
unnormalized transposed context). Denominators for batches {0,1} are
all-reduced as soon as batch 1's scores finish (~1/3 into the kernel) and
{2,3} right after batch 3's scores; the B-blocks and the first
out-projection half execute during the collectives' ~50us latency, so the
tensor engine never waits on the network.

The context is produced directly transposed ([dhead, q]) by using the V
projection as the stationary matmul operand two heads at a time (the
second head's queries occupy columns 100:200 of a block output), which
removes the 64 per-head transposes the combine step otherwise needs.
Normalization is then one broadcast reciprocal multiply per head pair.

LayerNorm gamma/beta are folded into the projection weights/biases on the
host; the V-projection bias is folded into the final output bias (exact
because softmax weights sum to one).
"""
import sys

sys.path.insert(0, "/opt/trn_rl_repo")

import numpy as np
import ml_dtypes

import concourse.bacc as bacc
import concourse.bass as bass
import concourse.mybir as mybir
import concourse.tile as tile
from concourse.bass_utils import run_bass_kernel_spmd
from concourse.masks import make_identity
from concourse import library_config

B, NQ, HW, D, H = 4, 100, 4096, 1024, 16
HD = D // H          # 64
NCORE = 8
KC = HW // NCORE     # 512 keys per core per batch
NKT = KC // 128      # 4 key sub-tiles of 128
NDC = D // 128       # 8 chunks of the model dim
NHP = H // 2         # 8 head pairs
EPS = 1e-5
SCALE = 1.0 / np.sqrt(np.float32(HD))  # 1/8

F32 = mybir.dt.float32
BF16 = mybir.dt.bfloat16
AF = mybir.ActivationFunctionType
ALU = mybir.AluOpType

_compiled = {}


def _build():
    nc = bacc.Bacc("TRN2", target_bir_lowering=False, num_devices=NCORE)

    kv_d = nc.dram_tensor("kv", [B, NKT, 128, D], BF16, kind="ExternalInput")
    q_d = nc.dram_tensor("q", [B, NQ, D], BF16, kind="ExternalInput")
    mask_d = nc.dram_tensor("maskT", [B, 128, NKT, NQ], BF16, kind="ExternalInput")
    wq_d = nc.dram_tensor("wqT", [128, NDC, D], BF16, kind="ExternalInput")
    wk_d = nc.dram_tensor("wkT", [128, NDC, D], BF16, kind="ExternalInput")
    wv_d = nc.dram_tensor("wvT", [128, NDC, D], BF16, kind="ExternalInput")
    wo_d = nc.dram_tensor("woT", [128, NDC, D], BF16, kind="ExternalInput")
    bq_d = nc.dram_tensor("biasq", [128, NDC], F32, kind="ExternalInput")
    bk_d = nc.dram_tensor("biask", [128, NDC], F32, kind="ExternalInput")
    out_d = nc.dram_tensor("out", [128, NDC, B, NQ], F32, kind="ExternalOutput")

    with tile.TileContext(nc) as tc:
        with (
            tc.tile_pool(name="sb", bufs=1) as sb,
            tc.tile_pool(name="ps", bufs=1, space="PSUM") as ps,
            tc.tile_pool(name="dram", bufs=1, space="DRAM") as dram,
        ):
            # ---- constants ----
            ident = sb.tile([128, 128], BF16, tag="ident")
            make_identity(nc, ident[:])
            # partition_broadcast lives in the gpsimd `attn` library;
            # make_identity's iota (standard lib) must run first.
            nc.gpsimd.load_library(library_config.attn)
            eps_t = sb.tile([128, 1], F32, tag="eps")
            nc.vector.memset(eps_t[:], EPS)
            ones_t = sb.tile([128, 1], BF16, tag="ones")
            nc.vector.memset(ones_t[:], 1.0)

            wk_sb = sb.tile([128, NDC, D], BF16, tag="wk")
            wv_sb = sb.tile([128, NDC, D], BF16, tag="wv")
            bqv_sb = sb.tile([128, NDC], F32, tag="bqv")
            bkv_sb = sb.tile([128, NDC], F32, tag="bkv")
            bq_sb = [bqv_sb[:, j:j + 1] for j in range(NDC)]
            bk_sb = [bkv_sb[:, j:j + 1] for j in range(NDC)]
            wq_sb = sb.tile([128, NDC, D], BF16, tag="wq", bufs=1, name="wq")
            wo_sb = sb.tile([128, NDC, D], BF16, tag="wo")

            def load_weights():
                nc.sync.dma_start(wk_sb[:], wk_d[:])
                nc.sync.dma_start(bkv_sb[:], bk_d[:])
                nc.sync.dma_start(wq_sb[:], wq_d[:])
                nc.sync.dma_start(bqv_sb[:], bq_d[:])
                nc.sync.dma_start(wv_sb[:], wv_d[:])
                nc.sync.dma_start(wo_sb[:], wo_d[:])

            def layernorm_to_bf16(x_f32, xn_bf16, p):
                """(x - mean) * rsqrt(var + eps), row-wise over the free dim."""
                stats = sb.tile([128, 2, 6], F32, tag="lnstats", bufs=4)
                nc.vector.bn_stats(stats[:p, 0, :], x_f32[:p, 0:512])
                nc.vector.bn_stats(stats[:p, 1, :], x_f32[:p, 512:1024])
                mv = sb.tile([128, 2], F32, tag="lnmv", bufs=4)
                nc.vector.bn_aggr(mv[:p], stats[:p])
                rstd = sb.tile([128, 1], F32, tag="lnrstd", bufs=4)
                nc.scalar.activation(rstd[:p], mv[:p, 1:2], AF.Sqrt, bias=eps_t[:p])
                nc.vector.reciprocal(rstd[:p], rstd[:p])
                nc.vector.tensor_scalar(
                    xn_bf16[:p], x_f32[:p], mv[:p, 0:1], rstd[:p],
                    ALU.subtract, ALU.mult,
                )

            # collective buffers (denominators, [2, H, NQ] per batch pair)
            sloc = [dram.tile([2, H, NQ], F32, tag=f"sloc{g}", name=f"sloc{g}") for g in range(2)]
            sglob = [dram.tile([2, H, NQ], F32, tag=f"sglob{g}", name=f"sglob{g}") for g in range(2)]
            ONE_AR = False
            if ONE_AR:
                sloc_all = dram.tile([B, H, NQ], F32, tag="slocall", name="sloc_all")
                sglob_all = dram.tile([B, H, NQ], F32, tag="sglob_all", name="sglob_all")
                sloc = [sloc_all[0:2], sloc_all[2:4]]
                sglob = [sglob_all[0:2], sglob_all[2:4]]

            # normalized, transposed context for all batches: [p, hp, b, q]
            ctxT_all = sb.tile([128, NDC, B, NQ], BF16, tag="ctxTall")
            qnT = sb.tile([128, NDC, B, NQ], BF16, tag="qnT")
            qpT = []

            def kv_lnt(b):
                """kv load + LayerNorm + transpose for batch b -> kvnT."""
                kvnT = sb.tile([128, NDC, NKT, 128], BF16, tag="kvnT", bufs=2,
                               name=f"kvnT_{b}")
                for r in range(NKT):
                    kvraw = sb.tile([128, D], BF16, tag="kvraw", bufs=3)
                    # batch 0 leads the sync queue; later batches stream on
                    # the scalar DGE queue so they bypass the weight loads
                    (nc.sync if b == 0 else nc.scalar).dma_start(
                        kvraw[:], kv_d[b, r])
                    xn = sb.tile([128, D], BF16, tag="xn", bufs=2)
                    layernorm_to_bf16(kvraw, xn, 128)
                    for k4 in range(NDC // 4):
                        tr = ps.tile([128, 4, 128], BF16, tag="small", bufs=2)
                        for kk in range(4):
                            k = 4 * k4 + kk
                            nc.tensor.transpose(
                                tr[:, kk, :], xn[:, k * 128:(k + 1) * 128], ident[:]
                            )
                        nc.vector.tensor_copy(
                            out=kvnT[:, 4 * k4:4 * k4 + 4, r, :], in_=tr[:]
                        )
                return kvnT

            def kproj(b, kvnT):
                """K projection -> kpT[j]: [128 dout, KC keys] (+bias)."""
                kpT = []
                for j in range(NDC):
                    kpT.append(
                        sb.tile([128, KC], BF16, tag=f"kpT{j}", bufs=1,
                                name=f"kpT{j}_{b}")
                    )
                    acc = ps.tile([128, KC], F32, tag="mm", bufs=2)
                    for k in range(NDC):
                        nc.tensor.matmul(
                            acc[:],
                            lhsT=wk_sb[:, k, j * 128:(j + 1) * 128],
                            rhs=kvnT[:, k, :, :].rearrange("p r k -> p (r k)"),
                            start=(k == 0), stop=(k == NDC - 1),
                        )
                    nc.scalar.activation(
                        kpT[j][:], acc[:], AF.Identity, bias=bk_sb[j][:]
                    )
                return kpT

            def q_pipeline():
                """LayerNorm + transpose + projection of q, all batches."""
                for b in range(B):
                    qraw = sb.tile([NQ, D], BF16, tag="qraw", bufs=2)
                    nc.scalar.dma_start(qraw[:], q_d[b])
                    qn = sb.tile([NQ, D], BF16, tag="qn", bufs=2)
                    layernorm_to_bf16(qraw, qn, NQ)
                    for k4 in range(NDC // 4):
                        tr = ps.tile([128, 4, NQ], BF16, tag="small", bufs=2)
                        for kk in range(4):
                            k = 4 * k4 + kk
                            nc.tensor.transpose(
                                tr[:, kk, :], qn[:, k * 128:(k + 1) * 128],
                                ident[:NQ, :NQ],
                            )
                        nc.vector.tensor_copy(
                            out=qnT[:, 4 * k4:4 * k4 + 4, b, :], in_=tr[:]
                        )
                # qpT_pad[j]: [128, B, 2, NQ] block-diagonal by head so the
                # scores matmul can use the full K=128 contraction for two
                # heads at once: rows 0:64 hold head 2j over i=0 columns,
                # rows 64:128 hold head 2j+1 over i=1 columns, zeros elsewhere.
                for j in range(NDC):
                    qpT.append(
                        sb.tile([128, B, 2, NQ], BF16, tag=f"qpT{j}",
                                name=f"qpT{j}")
                    )
                    nc.gpsimd.memset(qpT[j][:], 0.0)
                    acc = ps.tile([128, B * NQ], F32, tag="sc", bufs=2)
                    for k in range(NDC):
                        nc.tensor.matmul(
                            acc[:],
                            lhsT=wq_sb[:, k, j * 128:(j + 1) * 128],
                            rhs=qnT[:, k, :, :].rearrange("p b q -> p (b q)"),
                            start=(k == 0), stop=(k == NDC - 1),
                        )
                    nc.scalar.activation(
                        qpT[j][0:HD, :, 0, :],
                        acc[0:HD, :].rearrange("p (b q) -> p b q", b=B),
                        AF.Identity, bias=bq_sb[j][0:HD],
                    )
                    nc.scalar.activation(
                        qpT[j][HD:128, :, 1, :],
                        acc[HD:128, :].rearrange("p (b q) -> p b q", b=B),
                        AF.Identity, bias=bq_sb[j][HD:128],
                    )

            def scores_exp_den(b, kpT):
                """scores^T, exp, mask, denominators + sloc DMA for batch b.
                Returns exp_all (masked exp, kept for the deferred ctx)."""
                mask_b = sb.tile([128, NKT, NQ], BF16, tag="maskb", bufs=2)
                nc.scalar.dma_start(mask_b[:], mask_d[b])
                exp_all = sb.tile([128, NKT, H, NQ], BF16, tag="expall", bufs=2,
                                  name=f"exp_{b}")
                for j in range(NDC):
                    # scores^T for heads 2j, 2j+1 together: full-K matmuls
                    # against the block-diagonal qpT_pad, N = 2*NQ.
                    for c2 in range(2):
                        sc = ps.tile([128, 2, 2, NQ], F32, tag="sc", bufs=2)
                        for cc in range(2):
                            c = 2 * c2 + cc
                            nc.tensor.matmul(
                                sc[:, cc, :, :].rearrange("p i q -> p (i q)"),
                                lhsT=kpT[j][:, c * 128:(c + 1) * 128],
                                rhs=qpT[j][:, b, :, :].rearrange(
                                    "p i q -> p (i q)"),
                                start=True, stop=True,
                            )
                        nc.scalar.activation(
                            exp_all[:, 2 * c2:2 * c2 + 2, 2 * j:2 * j + 2, :],
                            sc[:], AF.Exp, scale=float(SCALE),
                        )
                        # mask applied per tile so the denominators can
                        # follow each head-quartet immediately
                        for hh in range(2):
                            nc.vector.tensor_mul(
                                exp_all[:, 2 * c2:2 * c2 + 2, 2 * j + hh, :],
                                exp_all[:, 2 * c2:2 * c2 + 2, 2 * j + hh, :],
                                mask_b[:, 2 * c2:2 * c2 + 2, :],
                            )
                    if j % 2 == 1:
                        # denominators for heads 2j-2..2j+2: ones-vector
                        # matmuls summing the 512 local keys, then a copy to
                        # SBUF (DMA can't read PSUM) and the sloc quarter DMA.
                        qt = j // 2
                        den_ps = ps.tile([1, 512], F32, tag="sc", bufs=2)
                        for c in range(NKT):
                            nc.tensor.matmul(
                                den_ps[:, 0:400],
                                lhsT=ones_t[:],
                                rhs=exp_all[:, c, 4 * qt:4 * qt + 4, :]
                                .rearrange("p h q -> p (h q)"),
                                start=(c == 0), stop=(c == NKT - 1),
                            )
                        den_sb = sb.tile([1, 400], F32, tag="densb", bufs=2)
                        nc.scalar.copy(den_sb[:], den_ps[:, 0:400])
                        nc.gpsimd.dma_start(
                            sloc[b // 2][b % 2:b % 2 + 1, 4 * qt:4 * qt + 4, :]
                            .rearrange("b h q -> b (h q)"),
                            den_sb[:],
                        )
                return exp_all

            def vproj(b, kvnT):
                """V projection -> vp[r]: [128 keys, H, HD]."""
                vp = []
                for r in range(NKT):
                    vpe = sb.tile([128, H, HD], BF16, tag=f"vpe{r}", bufs=1,
                                  name=f"vpe{r}_{b}")
                    vp.append(vpe)
                    for nh in range(2):
                        acc = ps.tile([128, 512], F32, tag="mmv", bufs=2)
                        for k in range(NDC):
                            nc.tensor.matmul(
                                acc[:],
                                lhsT=kvnT[:, k, r, :],
                                rhs=wv_sb[:, k, nh * 512:(nh + 1) * 512],
                                start=(k == 0), stop=(k == NDC - 1),
                            )
                        nc.scalar.copy(
                            vp[r][:, nh * 8:(nh + 1) * 8, :],
                            acc[:].rearrange("p (g d) -> p g d", g=8),
                        )
                return vp

            def ctx_block(b, exp_all, vp):
                """Unnormalized transposed context for batch b.

                Per head pair hp, lhsT = [vp_h | vp_h1] ([128, 128]), rhs =
                [exp_h | exp_h1] ([128, 200]); the diagonal blocks of the
                [128, 200] output are the two heads' ctx^T, the off-diagonal
                blocks are discarded."""
                ctxTu = sb.tile([128, NHP, NQ], BF16, tag="ctxTu", bufs=4,
                                name=f"ctxTu_{b}")
                for hp in range(NHP):
                    cps = ps.tile([128, 2, NQ], F32, tag="mmv", bufs=2)
                    for c in range(NKT):
                        nc.tensor.matmul(
                            cps[:].rearrange("p i q -> p (i q)"),
                            lhsT=vp[c][:, 2 * hp:2 * hp + 2, :].rearrange(
                                "p h d -> p (h d)"),
                            rhs=exp_all[:, c, 2 * hp:2 * hp + 2, :].rearrange(
                                "p h q -> p (h q)"),
                            start=(c == 0), stop=(c == NKT - 1),
                        )
                    nc.vector.tensor_copy(
                        out=ctxTu[0:HD, hp, :], in_=cps[0:HD, 0, :])
                    nc.vector.tensor_copy(
                        out=ctxTu[HD:128, hp, :], in_=cps[HD:128, 1, :])
                return ctxTu

            def recip_combine(b, ctxTu):
                """1/denominator broadcast + normalize into ctxT_all."""
                rr = sb.tile([1, H, NQ], F32, tag="rr", bufs=1)
                nc.gpsimd.dma_start(rr[:], sglob[b // 2][b % 2:b % 2 + 1])
                rbc = sb.tile([128, H, NQ], F32, tag="rbc", bufs=2)
                nc.gpsimd.partition_broadcast(rbc[:], rr[:])
                nc.vector.reciprocal_approx_fast(
                    out=rbc[:].rearrange("p h q -> p (h q)"),
                    in_=rbc[:].rearrange("p h q -> p (h q)"),
                )
                for hp in range(NHP):
                    nc.vector.tensor_mul(
                        ctxT_all[0:HD, hp, b, :], ctxTu[0:HD, hp, :],
                        rbc[0:HD, 2 * hp, :],
                    )
                    nc.vector.tensor_mul(
                        ctxT_all[HD:128, hp, b, :], ctxTu[HD:128, hp, :],
                        rbc[HD:128, 2 * hp + 1, :],
                    )

            # reuses wq's slot (wq is dead after the Q projection)
            out_sb = sb.tile([128, NDC, B, NQ], F32, tag="wq", bufs=1,
                             name="out_sb")

            def outproj_half(half):
                """Out-projection for a pair of batches (N = 2*NQ)."""
                b0 = 2 * half
                for m in range(NDC):
                    acc = ps.tile([128, 2 * NQ], F32, tag="sc", bufs=2)
                    for k in range(NDC):
                        nc.tensor.matmul(
                            acc[:],
                            lhsT=wo_sb[:, k, m * 128:(m + 1) * 128],
                            rhs=ctxT_all[:, k, b0:b0 + 2, :].rearrange(
                                "p b q -> p (b q)"),
                            start=(k == 0), stop=(k == NDC - 1),
                        )
                    nc.scalar.copy(
                        out_sb[:, m, b0:b0 + 2, :],
                        acc[:].rearrange("p (b q) -> p b q", b=2),
                    )
                    if m % 2 == 1:
                        nc.sync.dma_start(
                            out_d[:, m - 1:m + 1, b0:b0 + 2, :],
                            out_sb[:, m - 1:m + 1, b0:b0 + 2, :],
                        )

            # ---- pipelined schedule ----
            # The A-track (kv LN/transpose -> K proj -> scores -> exp*mask ->
            # denominators -> collective input) is the latency-critical chain
            # that gates the AllReduces; give it strict scheduler priority
            # over the B-track (V proj / ctx) filler work.
            kvnT0 = kv_lnt(0)
            load_weights()
            # warmup barrier: a tiny AllReduce fired at t~0 pays the
            # collective-firmware launch once and aligns the 8 cores long
            # before the denominator AllReduces, shrinking their skew wait.
            bar_i = dram.tile([1, 8], F32, tag="bar_i", name="bar_i")
            bar_o = dram.tile([1, 8], F32, tag="bar_o", name="bar_o")
            nc.gpsimd.collective_compute(
                "AllReduce", ALU.add,
                replica_groups=[list(range(NCORE))],
                ins=[bar_i[:].opt()], outs=[bar_o[:].opt()],
            )
            kpT0 = kproj(0, kvnT0)
            q_pipeline()
            exp0 = scores_exp_den(0, kpT0)

            kvnT1 = kv_lnt(1)
            kpT1 = kproj(1, kvnT1)
            exp1 = scores_exp_den(1, kpT1)

            if not ONE_AR:
                nc.gpsimd.collective_compute(
                    "AllReduce", ALU.add,
                    replica_groups=[list(range(NCORE))],
                    ins=[sloc[0][:].opt()], outs=[sglob[0][:].opt()],
                )

            ctxTu0 = ctx_block(0, exp0, vproj(0, kvnT0))

            kvnT2 = kv_lnt(2)
            kpT2 = kproj(2, kvnT2)
            exp2 = scores_exp_den(2, kpT2)

            ctxTu1 = ctx_block(1, exp1, vproj(1, kvnT1))

            kvnT3 = kv_lnt(3)
            kpT3 = kproj(3, kvnT3)
            # the sed3 chain gates AR2 (the last collective): give its
            # cross-engine ping-pong (bias/exp/mask/den) a scheduling edge
            # over concurrent B-track copies.
            with tc.high_priority(offset=50_000):
                exp3 = scores_exp_den(3, kpT3)

            if ONE_AR:
                nc.gpsimd.collective_compute(
                    "AllReduce", ALU.add,
                    replica_groups=[list(range(NCORE))],
                    ins=[sloc_all[:].opt()], outs=[sglob_all[:].opt()],
                )
            else:
                nc.gpsimd.collective_compute(
                    "AllReduce", ALU.add,
                    replica_groups=[list(range(NCORE))],
                    ins=[sloc[1][:].opt()], outs=[sglob[1][:].opt()],
                )

            ctxTu2 = ctx_block(2, exp2, vproj(2, kvnT2))
            ctxTu3 = ctx_block(3, exp3, vproj(3, kvnT3))

            recip_combine(0, ctxTu0)
            recip_combine(1, ctxTu1)
            outproj_half(0)
            recip_combine(2, ctxTu2)
            recip_combine(3, ctxTu3)
            outproj_half(1)

    nc.compile()
    return nc


def _prep_in_maps(q, kv, mask, in_proj_w, in_proj_b, out_w, out_b,
                  g_q, b_q, g_kv, b_kv):
    """Host-side prep: fold LN affine + V-bias, shard kv/mask per core.

    Returns (in_maps, bias_total)."""
    q = np.asarray(q, np.float32)
    kv = np.asarray(kv, np.float32)
    mask = np.asarray(mask)
    in_proj_w = np.asarray(in_proj_w, np.float32)
    in_proj_b = np.asarray(in_proj_b, np.float32)
    out_w = np.asarray(out_w, np.float32)
    out_b = np.asarray(out_b, np.float32)
    g_q = np.asarray(g_q, np.float32)
    b_q = np.asarray(b_q, np.float32)
    g_kv = np.asarray(g_kv, np.float32)
    b_kv = np.asarray(b_kv, np.float32)

    Wq, Wk, Wv = in_proj_w[:D], in_proj_w[D:2 * D], in_proj_w[2 * D:]
    bq, bk, bv = in_proj_b[:D], in_proj_b[D:2 * D], in_proj_b[2 * D:]

    # Fold LayerNorm affine into projections: LN(x)*g+b @ W^T + c
    #   = LN(x) @ (W*g)^T + (W@b + c)
    WqT = (Wq * g_q[None, :]).T.astype(ml_dtypes.bfloat16)
    WkT = (Wk * g_kv[None, :]).T.astype(ml_dtypes.bfloat16)
    WvT = (Wv * g_kv[None, :]).T.astype(ml_dtypes.bfloat16)
    bq_eff = (bq + Wq @ b_q).astype(np.float32)
    bk_eff = (bk + Wk @ b_kv).astype(np.float32)
    bv_eff = (bv + Wv @ b_kv).astype(np.float32)
    # V bias passes through softmax unchanged (weights sum to 1): fold into
    # the final output bias on the host.
    WoT = out_w.T.astype(ml_dtypes.bfloat16)
    bias_total = (out_b + out_w @ bv_eff).astype(np.float32)

    # per-query key mask; all-zero mask rows attend everywhere
    kv16 = kv.astype(ml_dtypes.bfloat16)
    allowed = (mask != 0)
    has_any = allowed.any(axis=-1, keepdims=True)
    eff = np.where(has_any, allowed, True)  # [B, NQ, HW] bool

    common = {
        "q": np.ascontiguousarray(q.astype(ml_dtypes.bfloat16)),
        "wqT": np.ascontiguousarray(WqT.reshape(NDC, 128, D).transpose(1, 0, 2)),
        "wkT": np.ascontiguousarray(WkT.reshape(NDC, 128, D).transpose(1, 0, 2)),
        "wvT": np.ascontiguousarray(WvT.reshape(NDC, 128, D).transpose(1, 0, 2)),
        "woT": np.ascontiguousarray(WoT.reshape(NDC, 128, D).transpose(1, 0, 2)),
        "biasq": np.ascontiguousarray(bq_eff.reshape(NDC, 128).T),
        "biask": np.ascontiguousarray(bk_eff.reshape(NDC, 128).T),
    }
    in_maps = []
    for c in range(NCORE):
        sl = slice(c * KC, (c + 1) * KC)
        kv_c = kv16[:, sl, :].reshape(B, NKT, 128, D)
        # mask slice -> [B, 128, NKT, NQ] bf16 (keysub-tile on partitions)
        m_c = eff[:, :, sl].transpose(0, 2, 1).reshape(B, NKT, 128, NQ)
        m_c = m_c.transpose(0, 2, 1, 3).astype(ml_dtypes.bfloat16)
        in_maps.append({
            **common,
            "kv": np.ascontiguousarray(kv_c),
            "maskT": np.ascontiguousarray(m_c),
        })
    return in_maps, bias_total


def kernel(q, kv, mask, in_proj_w, in_proj_b, out_w, out_b, g_q, b_q, g_kv, b_kv):
    in_maps, bias_total = _prep_in_maps(
        q, kv, mask, in_proj_w, in_proj_b, out_w, out_b, g_q, b_q, g_kv, b_kv
    )
    if "nc" not in _compiled:
        _compiled["nc"] = _build()
    nc = _compiled["nc"]

    res = run_bass_kernel_spmd(nc, in_maps, core_ids=list(range(NCORE)))

    out = np.zeros((B, NQ, D), np.float32)
    for c in range(NCORE):
        part = res.results[c]["out"]  # [128 p, NDC m, B, NQ]; dout = m*128+p
        out += part.transpose(2, 3, 1, 0).reshape(B, NQ, D)
    out += bias_total[None, None, :]
    return out


# revision 23
# speedup vs baseline: 1.0293x; 1.0293x over previous
"""Masked cross-attention (B=4, NQ=100, HW=4096, D=1024, H=16) on 8 TRN2 cores.

Sharding: kv rows (keys) are split 8 ways; each core runs LayerNorm + K/V
projection on its 512-key slice per batch, computes unnormalized partial
attention for all (b, h) against its keys, all-reduces the softmax
denominators on device, normalizes, and computes a partial out-projection.
The host sums the 8 partial outputs and adds the folded bias.

Schedule: the kernel is split into per-batch A-blocks (kv LN/transpose,
K-projection, scores, exp*mask, denominators) and B-blocks (V-projection,
unnormalized transposed context). Denominators for batches {0,1} are
all-reduced as soon as batch 1's scores finish (~1/3 into the kernel) and
{2,3} right after batch 3's scores; the B-blocks and the first
out-projection half execute during the collectives' ~50us latency, so the
tensor engine never waits on the network.

The context is produced directly transposed ([dhead, q]) by using the V
projection as the stationary matmul operand two heads at a time (the
second head's queries occupy columns 100:200 of a block output), which
removes the 64 per-head transposes the combine step otherwise needs.
Normalization is then one broadcast reciprocal multiply per head pair.

LayerNorm gamma/beta are folded into the projection weights/biases on the
host; the V-projection bias is folded into the final output bias (exact
because softmax weights sum to one).
"""
import sys

sys.path.insert(0, "/opt/trn_rl_repo")

import numpy as np
import ml_dtypes

import concourse.bacc as bacc
import concourse.bass as bass
import concourse.mybir as mybir
import concourse.tile as tile
from concourse.bass_utils import run_bass_kernel_spmd
from concourse.masks import make_identity
from concourse import library_config

B, NQ, HW, D, H = 4, 100, 4096, 1024, 16
HD = D // H          # 64
NCORE = 8
KC = HW // NCORE     # 512 keys per core per batch
NKT = KC // 128      # 4 key sub-tiles of 128
NDC = D // 128       # 8 chunks of the model dim
NHP = H // 2         # 8 head pairs
EPS = 1e-5
SCALE = 1.0 / np.sqrt(np.float32(HD))  # 1/8

F32 = mybir.dt.float32
BF16 = mybir.dt.bfloat16
AF = mybir.ActivationFunctionType
ALU = mybir.AluOpType

_compiled = {}


def _build():
    nc = bacc.Bacc("TRN2", target_bir_lowering=False, num_devices=NCORE)

    kv_d = nc.dram_tensor("kv", [B, NKT, 128, D], BF16, kind="ExternalInput")
    q_d = nc.dram_tensor("q", [B, NQ, D], BF16, kind="ExternalInput")
    mask_d = nc.dram_tensor("maskT", [B, 128, NKT, NQ], BF16, kind="ExternalInput")
    wq_d = nc.dram_tensor("wqT", [128, NDC, D], BF16, kind="ExternalInput")
    wk_d = nc.dram_tensor("wkT", [128, NDC, D], BF16, kind="ExternalInput")
    wv_d = nc.dram_tensor("wvT", [128, NDC, D], BF16, kind="ExternalInput")
    wo_d = nc.dram_tensor("woT", [128, NDC, D], BF16, kind="ExternalInput")
    bq_d = nc.dram_tensor("biasq", [128, NDC], F32, kind="ExternalInput")
    bk_d = nc.dram_tensor("biask", [128, NDC], F32, kind="ExternalInput")
    out_d = nc.dram_tensor("out", [128, NDC, B, NQ], F32, kind="ExternalOutput")

    with tile.TileContext(nc) as tc:
        with (
            tc.tile_pool(name="sb", bufs=1) as sb,
            tc.tile_pool(name="ps", bufs=1, space="PSUM") as ps,
            tc.tile_pool(name="dram", bufs=1, space="DRAM") as dram,
        ):
            # ---- constants ----
            ident = sb.tile([128, 128], BF16, tag="ident")
            make_identity(nc, ident[:])
            # partition_broadcast lives in the gpsimd `attn` library;
            # make_identity's iota (standard lib) must run first.
            nc.gpsimd.load_library(library_config.attn)
            eps_t = sb.tile([128, 1], F32, tag="eps")
            nc.vector.memset(eps_t[:], EPS)
            ones_t = sb.tile([128, 1], BF16, tag="ones")
            nc.vector.memset(ones_t[:], 1.0)

            wk_sb = sb.tile([128, NDC, D], BF16, tag="wk")
            wv_sb = sb.tile([128, NDC, D], BF16, tag="wv")
            bqv_sb = sb.tile([128, NDC], F32, tag="bqv")
            bkv_sb = sb.tile([128, NDC], F32, tag="bkv")
            bq_sb = [bqv_sb[:, j:j + 1] for j in range(NDC)]
            bk_sb = [bkv_sb[:, j:j + 1] for j in range(NDC)]
            wq_sb = sb.tile([128, NDC, D], BF16, tag="wq", bufs=1, name="wq")
            wo_sb = sb.tile([128, NDC, D], BF16, tag="wo")

            def load_w(*pairs):
                for sb_t, d_t in pairs:
                    nc.sync.dma_start(sb_t[:], d_t[:])

            def layernorm_to_bf16(x_f32, xn_bf16, p):
                """(x - mean) * rsqrt(var + eps), row-wise over the free dim."""
                stats = sb.tile([128, 2, 6], F32, tag="lnstats", bufs=4)
                nc.vector.bn_stats(stats[:p, 0, :], x_f32[:p, 0:512])
                nc.vector.bn_stats(stats[:p, 1, :], x_f32[:p, 512:1024])
                mv = sb.tile([128, 2], F32, tag="lnmv", bufs=4)
                nc.vector.bn_aggr(mv[:p], stats[:p])
                rstd = sb.tile([128, 1], F32, tag="lnrstd", bufs=4)
                nc.scalar.activation(rstd[:p], mv[:p, 1:2], AF.Sqrt, bias=eps_t[:p])
                nc.vector.reciprocal(rstd[:p], rstd[:p])
                nc.vector.tensor_scalar(
                    xn_bf16[:p], x_f32[:p], mv[:p, 0:1], rstd[:p],
                    ALU.subtract, ALU.mult,
                )

            # collective buffers (denominators, [2, H, NQ] per batch pair)
            sloc = [dram.tile([2, H, NQ], F32, tag=f"sloc{g}", name=f"sloc{g}") for g in range(2)]
            sglob = [dram.tile([2, H, NQ], F32, tag=f"sglob{g}", name=f"sglob{g}") for g in range(2)]
            ONE_AR = False
            if ONE_AR:
                sloc_all = dram.tile([B, H, NQ], F32, tag="slocall", name="sloc_all")
                sglob_all = dram.tile([B, H, NQ], F32, tag="sglob_all", name="sglob_all")
                sloc = [sloc_all[0:2], sloc_all[2:4]]
                sglob = [sglob_all[0:2], sglob_all[2:4]]

            # normalized, transposed context for all batches: [p, hp, b, q]
            ctxT_all = sb.tile([128, NDC, B, NQ], BF16, tag="ctxTall")
            qnT = sb.tile([128, NDC, B, NQ], BF16, tag="qnT")
            qpT = []

            def kv_lnt(b):
                """kv load + LayerNorm + transpose for batch b -> kvnT."""
                kvnT = sb.tile([128, NDC, NKT, 128], BF16, tag="kvnT", bufs=2,
                               name=f"kvnT_{b}")
                for r in range(NKT):
                    kvraw = sb.tile([128, D], BF16, tag="kvraw", bufs=3)
                    nc.sync.dma_start(kvraw[:], kv_d[b, r])
                    xn = sb.tile([128, D], BF16, tag="xn", bufs=2)
                    layernorm_to_bf16(kvraw, xn, 128)
                    for k4 in range(NDC // 4):
                        tr = ps.tile([128, 4, 128], BF16, tag="small", bufs=2)
                        for kk in range(4):
                            k = 4 * k4 + kk
                            nc.tensor.transpose(
                                tr[:, kk, :], xn[:, k * 128:(k + 1) * 128], ident[:]
                            )
                        nc.vector.tensor_copy(
                            out=kvnT[:, 4 * k4:4 * k4 + 4, r, :], in_=tr[:]
                        )
                return kvnT

            def kproj(b, kvnT):
                """K projection -> kpT[j]: [128 dout, KC keys] (+bias)."""
                kpT = []
                for j in range(NDC):
                    kpT.append(
                        sb.tile([128, KC], BF16, tag=f"kpT{j}", bufs=1,
                                name=f"kpT{j}_{b}")
                    )
                    acc = ps.tile([128, KC], F32, tag="mm", bufs=2)
                    for k in range(NDC):
                        nc.tensor.matmul(
                            acc[:],
                            lhsT=wk_sb[:, k, j * 128:(j + 1) * 128],
                            rhs=kvnT[:, k, :, :].rearrange("p r k -> p (r k)"),
                            start=(k == 0), stop=(k == NDC - 1),
                        )
                    nc.scalar.activation(
                        kpT[j][:], acc[:], AF.Identity, bias=bk_sb[j][:]
                    )
                return kpT

            def q_pipeline_ln():
                """LayerNorm + transpose of q, all batches."""
                for b in range(B):
                    qraw = sb.tile([NQ, D], BF16, tag="qraw", bufs=2)
                    nc.sync.dma_start(qraw[:], q_d[b])
                    qn = sb.tile([NQ, D], BF16, tag="qn", bufs=2)
                    layernorm_to_bf16(qraw, qn, NQ)
                    for k4 in range(NDC // 4):
                        tr = ps.tile([128, 4, NQ], BF16, tag="small", bufs=2)
                        for kk in range(4):
                            k = 4 * k4 + kk
                            nc.tensor.transpose(
                                tr[:, kk, :], qn[:, k * 128:(k + 1) * 128],
                                ident[:NQ, :NQ],
                            )
                        nc.vector.tensor_copy(
                            out=qnT[:, 4 * k4:4 * k4 + 4, b, :], in_=tr[:]
                        )
            def q_pipeline_proj():
                # qpT_pad[j]: [128, B, 2, NQ] block-diagonal by head so the
                # scores matmul can use the full K=128 contraction for two
                # heads at once: rows 0:64 hold head 2j over i=0 columns,
                # rows 64:128 hold head 2j+1 over i=1 columns, zeros elsewhere.
                for j in range(NDC):
                    qpT.append(
                        sb.tile([128, B, 2, NQ], BF16, tag=f"qpT{j}",
                                name=f"qpT{j}")
                    )
                    nc.gpsimd.memset(qpT[j][:], 0.0)
                    acc = ps.tile([128, B * NQ], F32, tag="sc", bufs=2)
                    for k in range(NDC):
                        nc.tensor.matmul(
                            acc[:],
                            lhsT=wq_sb[:, k, j * 128:(j + 1) * 128],
                            rhs=qnT[:, k, :, :].rearrange("p b q -> p (b q)"),
                            start=(k == 0), stop=(k == NDC - 1),
                        )
                    nc.scalar.activation(
                        qpT[j][0:HD, :, 0, :],
                        acc[0:HD, :].rearrange("p (b q) -> p b q", b=B),
                        AF.Identity, bias=bq_sb[j][0:HD],
                    )
                    nc.scalar.activation(
                        qpT[j][HD:128, :, 1, :],
                        acc[HD:128, :].rearrange("p (b q) -> p b q", b=B),
                        AF.Identity, bias=bq_sb[j][HD:128],
                    )

            def scores_exp_den(b, kpT):
                """scores^T, exp, mask, denominators + sloc DMA for batch b.
                Returns exp_all (masked exp, kept for the deferred ctx)."""
                mask_b = sb.tile([128, NKT, NQ], BF16, tag="maskb", bufs=2)
                nc.sync.dma_start(mask_b[:], mask_d[b])
                exp_all = sb.tile([128, NKT, H, NQ], BF16, tag="expall", bufs=2,
                                  name=f"exp_{b}")
                for j in range(NDC):
                    # scores^T for heads 2j, 2j+1 together: full-K matmuls
                    # against the block-diagonal qpT_pad, N = 2*NQ.
                    for c2 in range(2):
                        sc = ps.tile([128, 2, 2, NQ], F32, tag="sc", bufs=2)
                        for cc in range(2):
                            c = 2 * c2 + cc
                            nc.tensor.matmul(
                                sc[:, cc, :, :].rearrange("p i q -> p (i q)"),
                                lhsT=kpT[j][:, c * 128:(c + 1) * 128],
                                rhs=qpT[j][:, b, :, :].rearrange(
                                    "p i q -> p (i q)"),
                                start=True, stop=True,
                            )
                        nc.scalar.activation(
                            exp_all[:, 2 * c2:2 * c2 + 2, 2 * j:2 * j + 2, :],
                            sc[:], AF.Exp, scale=float(SCALE),
                        )
                        # mask applied per tile so the denominators can
                        # follow each head-quartet immediately
                        for hh in range(2):
                            nc.vector.tensor_mul(
                                exp_all[:, 2 * c2:2 * c2 + 2, 2 * j + hh, :],
                                exp_all[:, 2 * c2:2 * c2 + 2, 2 * j + hh, :],
                                mask_b[:, 2 * c2:2 * c2 + 2, :],
                            )
                    if j % 2 == 1:
                        # denominators for heads 2j-2..2j+2: ones-vector
                        # matmuls summing the 512 local keys, then a copy to
                        # SBUF (DMA can't read PSUM) and the sloc quarter DMA.
                        qt = j // 2
                        den_ps = ps.tile([1, 512], F32, tag="sc", bufs=2)
                        for c in range(NKT):
                            nc.tensor.matmul(
                                den_ps[:, 0:400],
                                lhsT=ones_t[:],
                                rhs=exp_all[:, c, 4 * qt:4 * qt + 4, :]
                                .rearrange("p h q -> p (h q)"),
                                start=(c == 0), stop=(c == NKT - 1),
                            )
                        den_sb = sb.tile([1, 400], F32, tag="densb", bufs=2)
                        nc.scalar.copy(den_sb[:], den_ps[:, 0:400])
                        nc.gpsimd.dma_start(
                            sloc[b // 2][b % 2:b % 2 + 1, 4 * qt:4 * qt + 4, :]
                            .rearrange("b h q -> b (h q)"),
                            den_sb[:],
                        )
                return exp_all

            def vproj(b, kvnT):
                """V projection -> vp[r]: [128 keys, H, HD]."""
                vp = []
                for r in range(NKT):
                    vpe = sb.tile([128, H, HD], BF16, tag=f"vpe{r}", bufs=1,
                                  name=f"vpe{r}_{b}")
                    vp.append(vpe)
                    for nh in range(2):
                        acc = ps.tile([128, 512], F32, tag="mmv", bufs=2)
                        for k in range(NDC):
                            nc.tensor.matmul(
                                acc[:],
                                lhsT=kvnT[:, k, r, :],
                                rhs=wv_sb[:, k, nh * 512:(nh + 1) * 512],
                                start=(k == 0), stop=(k == NDC - 1),
                            )
                        nc.scalar.copy(
                            vp[r][:, nh * 8:(nh + 1) * 8, :],
                            acc[:].rearrange("p (g d) -> p g d", g=8),
                        )
                return vp

            def ctx_block(b, exp_all, vp):
                """Unnormalized transposed context for batch b.

                Per head pair hp, lhsT = [vp_h | vp_h1] ([128, 128]), rhs =
                [exp_h | exp_h1] ([128, 200]); the diagonal blocks of the
                [128, 200] output are the two heads' ctx^T, the off-diagonal
                blocks are discarded."""
                ctxTu = sb.tile([128, NHP, NQ], BF16, tag="ctxTu", bufs=4,
                                name=f"ctxTu_{b}")
                for hp in range(NHP):
                    cps = ps.tile([128, 2, NQ], F32, tag="mmv", bufs=2)
                    for c in range(NKT):
                        nc.tensor.matmul(
                            cps[:].rearrange("p i q -> p (i q)"),
                            lhsT=vp[c][:, 2 * hp:2 * hp + 2, :].rearrange(
                                "p h d -> p (h d)"),
                            rhs=exp_all[:, c, 2 * hp:2 * hp + 2, :].rearrange(
                                "p h q -> p (h q)"),
                            start=(c == 0), stop=(c == NKT - 1),
                        )
                    nc.vector.tensor_copy(
                        out=ctxTu[0:HD, hp, :], in_=cps[0:HD, 0, :])
                    nc.vector.tensor_copy(
                        out=ctxTu[HD:128, hp, :], in_=cps[HD:128, 1, :])
                return ctxTu

            def recip_combine(b, ctxTu):
                """1/denominator broadcast + normalize into ctxT_all."""
                rr = sb.tile([1, H, NQ], F32, tag="rr", bufs=1)
                nc.gpsimd.dma_start(rr[:], sglob[b // 2][b % 2:b % 2 + 1])
                rbc = sb.tile([128, H, NQ], F32, tag="rbc", bufs=2)
                nc.gpsimd.partition_broadcast(rbc[:], rr[:])
                nc.vector.reciprocal_approx_fast(
                    out=rbc[:].rearrange("p h q -> p (h q)"),
                    in_=rbc[:].rearrange("p h q -> p (h q)"),
                )
                for hp in range(NHP):
                    nc.vector.tensor_mul(
                        ctxT_all[0:HD, hp, b, :], ctxTu[0:HD, hp, :],
                        rbc[0:HD, 2 * hp, :],
                    )
                    nc.vector.tensor_mul(
                        ctxT_all[HD:128, hp, b, :], ctxTu[HD:128, hp, :],
                        rbc[HD:128, 2 * hp + 1, :],
                    )

            # reuses wq's slot (wq is dead after the Q projection)
            out_sb = sb.tile([128, NDC, B, NQ], F32, tag="wq", bufs=1,
                             name="out_sb")

            def outproj_half(half):
                """Out-projection for a pair of batches (N = 2*NQ)."""
                b0 = 2 * half
                for m in range(NDC):
                    acc = ps.tile([128, 2 * NQ], F32, tag="sc", bufs=2)
                    for k in range(NDC):
                        nc.tensor.matmul(
                            acc[:],
                            lhsT=wo_sb[:, k, m * 128:(m + 1) * 128],
                            rhs=ctxT_all[:, k, b0:b0 + 2, :].rearrange(
                                "p b q -> p (b q)"),
                            start=(k == 0), stop=(k == NDC - 1),
                        )
                    nc.scalar.copy(
                        out_sb[:, m, b0:b0 + 2, :],
                        acc[:].rearrange("p (b q) -> p b q", b=2),
                    )
                    if m % 2 == 1:
                        nc.sync.dma_start(
                            out_d[:, m - 1:m + 1, b0:b0 + 2, :],
                            out_sb[:, m - 1:m + 1, b0:b0 + 2, :],
                        )

            # ---- pipelined schedule ----
            # The A-track (kv LN/transpose -> K proj -> scores -> exp*mask ->
            # denominators -> collective input) is the latency-critical chain
            # that gates the AllReduces; give it strict scheduler priority
            # over the B-track (V proj / ctx) filler work.
            kvnT0 = kv_lnt(0)
            load_w((wk_sb, wk_d), (bkv_sb, bk_d))
            # warmup barrier: a tiny AllReduce fired at t~0 pays the
            # collective-firmware launch once and aligns the 8 cores long
            # before the denominator AllReduces, shrinking their skew wait.
            bar_i = dram.tile([1, 8], F32, tag="bar_i", name="bar_i")
            bar_o = dram.tile([1, 8], F32, tag="bar_o", name="bar_o")
            nc.gpsimd.collective_compute(
                "AllReduce", ALU.add,
                replica_groups=[list(range(NCORE))],
                ins=[bar_i[:].opt()], outs=[bar_o[:].opt()],
            )
            kpT0 = kproj(0, kvnT0)
            q_pipeline_ln()
            load_w((wq_sb, wq_d), (bqv_sb, bq_d))
            q_pipeline_proj()
            exp0 = scores_exp_den(0, kpT0)
            load_w((wv_sb, wv_d))

            kvnT1 = kv_lnt(1)
            kpT1 = kproj(1, kvnT1)
            exp1 = scores_exp_den(1, kpT1)
            load_w((wo_sb, wo_d))

            if not ONE_AR:
                nc.gpsimd.collective_compute(
                    "AllReduce", ALU.add,
                    replica_groups=[list(range(NCORE))],
                    ins=[sloc[0][:].opt()], outs=[sglob[0][:].opt()],
                )

            ctxTu0 = ctx_block(0, exp0, vproj(0, kvnT0))

            kvnT2 = kv_lnt(2)
            kpT2 = kproj(2, kvnT2)
            exp2 = scores_exp_den(2, kpT2)

            ctxTu1 = ctx_block(1, exp1, vproj(1, kvnT1))

            kvnT3 = kv_lnt(3)
            kpT3 = kproj(3, kvnT3)
            # the sed3 chain gates AR2 (the last collective): give its
            # cross-engine ping-pong (bias/exp/mask/den) a scheduling edge
            # over concurrent B-track copies.
            with tc.high_priority(offset=50_000):
                exp3 = scores_exp_den(3, kpT3)

            if ONE_AR:
                nc.gpsimd.collective_compute(
                    "AllReduce", ALU.add,
                    replica_groups=[list(range(NCORE))],
                    ins=[sloc_all[:].opt()], outs=[sglob_all[:].opt()],
                )
            else:
                nc.gpsimd.collective_compute(
                    "AllReduce", ALU.add,
                    replica_groups=[list(range(NCORE))],
                    ins=[sloc[1][:].opt()], outs=[sglob[1][:].opt()],
                )

            ctxTu2 = ctx_block(2, exp2, vproj(2, kvnT2))
            ctxTu3 = ctx_block(3, exp3, vproj(3, kvnT3))

            recip_combine(0, ctxTu0)
            recip_combine(1, ctxTu1)
            outproj_half(0)
            recip_combine(2, ctxTu2)
            recip_combine(3, ctxTu3)
            outproj_half(1)

    nc.compile()
    return nc


def _prep_in_maps(q, kv, mask, in_proj_w, in_proj_b, out_w, out_b,
                  g_q, b_q, g_kv, b_kv):
    """Host-side prep: fold LN affine + V-bias, shard kv/mask per core.

    Returns (in_maps, bias_total)."""
    q = np.asarray(q, np.float32)
    kv = np.asarray(kv, np.float32)
    mask = np.asarray(mask)
    in_proj_w = np.asarray(in_proj_w, np.float32)
    in_proj_b = np.asarray(in_proj_b, np.float32)
    out_w = np.asarray(out_w, np.float32)
    out_b = np.asarray(out_b, np.float32)
    g_q = np.asarray(g_q, np.float32)
    b_q = np.asarray(b_q, np.float32)
    g_kv = np.asarray(g_kv, np.float32)
    b_kv = np.asarray(b_kv, np.float32)

    Wq, Wk, Wv = in_proj_w[:D], in_proj_w[D:2 * D], in_proj_w[2 * D:]
    bq, bk, bv = in_proj_b[:D], in_proj_b[D:2 * D], in_proj_b[2 * D:]

    # Fold LayerNorm affine into projections: LN(x)*g+b @ W^T + c
    #   = LN(x) @ (W*g)^T + (W@b + c)
    WqT = (Wq * g_q[None, :]).T.astype(ml_dtypes.bfloat16)
    WkT = (Wk * g_kv[None, :]).T.astype(ml_dtypes.bfloat16)
    WvT = (Wv * g_kv[None, :]).T.astype(ml_dtypes.bfloat16)
    bq_eff = (bq + Wq @ b_q).astype(np.float32)
    bk_eff = (bk + Wk @ b_kv).astype(np.float32)
    bv_eff = (bv + Wv @ b_kv).astype(np.float32)
    # V bias passes through softmax unchanged (weights sum to 1): fold into
    # the final output bias on the host.
    WoT = out_w.T.astype(ml_dtypes.bfloat16)
    bias_total = (out_b + out_w @ bv_eff).astype(np.float32)

    # per-query key mask; all-zero mask rows attend everywhere
    kv16 = kv.astype(ml_dtypes.bfloat16)
    allowed = (mask != 0)
    has_any = allowed.any(axis=-1, keepdims=True)
    eff = np.where(has_any, allowed, True)  # [B, NQ, HW] bool

    common = {
        "q": np.ascontiguousarray(q.astype(ml_dtypes.bfloat16)),
        "wqT": np.ascontiguousarray(WqT.reshape(NDC, 128, D).transpose(1, 0, 2)),
        "wkT": np.ascontiguousarray(WkT.reshape(NDC, 128, D).transpose(1, 0, 2)),
        "wvT": np.ascontiguousarray(WvT.reshape(NDC, 128, D).transpose(1, 0, 2)),
        "woT": np.ascontiguousarray(WoT.reshape(NDC, 128, D).transpose(1, 0, 2)),
        "biasq": np.ascontiguousarray(bq_eff.reshape(NDC, 128).T),
        "biask": np.ascontiguousarray(bk_eff.reshape(NDC, 128).T),
    }
    in_maps = []
    for c in range(NCORE):
        sl = slice(c * KC, (c + 1) * KC)
        kv_c = kv16[:, sl, :].reshape(B, NKT, 128, D)
        # mask slice -> [B, 128, NKT, NQ] bf16 (keysub-tile on partitions)
        m_c = eff[:, :, sl].transpose(0, 2, 1).reshape(B, NKT, 128, NQ)
        m_c = m_c.transpose(0, 2, 1, 3).astype(ml_dtypes.bfloat16)
        in_maps.append({
            **common,
            "kv": np.ascontiguousarray(kv_c),
            "maskT": np.ascontiguousarray(m_c),
        })
    return in_maps, bias_total


def kernel(q, kv, mask, in_proj_w, in_proj_b, out_w, out_b, g_q, b_q, g_kv, b_kv):
    in_maps, bias_total = _prep_in_maps(
        q, kv, mask, in_proj_w, in_proj_b, out_w, out_b, g_q, b_q, g_kv, b_kv
    )
    if "nc" not in _compiled:
        _compiled["nc"] = _build()
    nc = _compiled["nc"]

    res = run_bass_kernel_spmd(nc, in_maps, core_ids=list(range(NCORE)))

    out = np.zeros((B, NQ, D), np.float32)
    for c in range(NCORE):
        part = res.results[c]["out"]  # [128 p, NDC m, B, NQ]; dout = m*128+p
        out += part.transpose(2, 3, 1, 0).reshape(B, NQ, D)
    out += bias_total[None, None, :]
    return out
